# revision 23
# baseline (speedup 1.0000x reference)
"""Chamfer distance kernel for Trainium2 (8 NeuronCores, SPMD).

Problem: input1 [B=4, N=8192, K=3], input2 [B=4, M=8192, K=3] (fp32).
  D[b,n,m] = ||input1[b,n] - input2[b,m]||
  out = mean_b( mean_m min_n D + mean_n min_m D )   (scalar fp32)

Strategy:
  - min(sqrt(x)) = sqrt(min(x)): mins on squared distances; sqrt at the end
    (host, 16k values per batch).
  - D^2 from one matmul via augmented coordinates (host-side prep),
    pre-scaled by sqrt(SCALE) so psum = SCALE * D^2 (keeps fp16 col
    accumulation clear of subnormals):
      W = g*[-2*a_x; -2*a_y; -2*a_z; ||a||^2; 1]   [5, n_half]  (stationary)
      R = g*[ b_x;    b_y;    b_z;   1; ||b||^2]   [5, M]       (moving)
      psum = W.T @ R = SCALE * D^2
  - K=5 contraction wastes 123/128 PE rows -> row-tile 4 concurrent
    matmuls via tile_position=(32g, 0); W/R data replicated on 4
    partition strips (0/32/64/96), each strip computing a different
    512-wide m-slice of a [128, 2048] psum chunk.
  - Sharding: 8 cores = 4 batches x 2 halves of N. Per core (4096 n's x
    all 8192 m's), per [128, 2048] psum chunk:
      DVE  tensor_reduce(min)  -> row-min entry       (PSUM read 1)
      ACT  copy psum -> SBUF fp16                     (PSUM read 2)
      GPS  tensor_tensor(min) fp16 -> col accumulator (SBUF)
    Host combines: partition-min + core-min + unscale + sqrt + means.
  - This walrus encodes at most ONE sync wait per TPB instruction;
    _split_multi_waits() hoists extra Tile-emitted waits onto NOPs.
"""

import numpy as np
from contextlib import ExitStack

B, N, M, K = 4, 8192, 8192, 3
NCORES = 8
NHALF = N // 2          # 4096 n's per core
P = 128                 # partitions
NB = NHALF // P         # 32 n-blocks per core
CW = 2048               # psum chunk width (4 banks, 4 row-tiled matmuls)
MC = M // CW            # 4 m-chunks
MMW = 512               # per-matmul moving width (1 bank)
SCALE = 4096.0          # psum carries SCALE * D^2

_cache = {}


def _build():
    import concourse.bass as bass
    import concourse.tile as tile
    from concourse import mybir

    f32 = mybir.dt.float32
    f16 = mybir.dt.float16
    amin = mybir.AluOpType.min
    W5 = NHALF + M  # columns of the wr operand plane

    nc = bass.Bass()
    wr_d = nc.declare_dram_parameter("wr", [5, W5], f32, isOutput=False)
    row_d = nc.declare_dram_parameter("row_out", [P, NB], f32, isOutput=True)
    col_d = nc.declare_dram_parameter("col_out", [P, M], f16, isOutput=True)

    with tile.TileContext(nc) as tc, ExitStack() as ctx:
        const = ctx.enter_context(tc.tile_pool(name="const", bufs=1))
        spool = ctx.enter_context(tc.tile_pool(name="spool", bufs=3))
        psum = ctx.enter_context(
            tc.tile_pool(name="psum", bufs=2, space="PSUM")
        )

        wr_s = const.tile([101, W5], f32)  # 4 replicas at strips 0/32/64/96
        colacc = const.tile([P, M], f16)
        rmins = const.tile([P, NB], f32)

        # load the operand plane onto all 4 partition strips straight from
        # DRAM: 8 parallel HWDGE DMAs (4 strips x 2 column halves)
        H5 = W5 // 2
        for g in range(4):
            for hh in range(2):
                nc.sync.dma_start(
                    wr_s[32 * g : 32 * g + 5, bass.ts(hh, H5)],
                    wr_d[:, bass.ts(hh, H5)],
                )

        def wsl(g, j):  # strip-g weights for n-block j
            return wr_s[32 * g : 32 * g + 5, bass.ts(j, P)]

        def rsl(g, q, t):  # strip-g moving operand, m-slice (q, t)
            return wr_s[32 * g : 32 * g + 5, bass.ds(NHALF + q * CW + t * MMW, MMW)]

        for j in range(NB):
            # full-width fp16 image of row-block j (filled by 4 ACT copies)
            s16 = spool.tile([P, M], f16, tag="s16")
            for q in range(MC):
                pt = psum.tile([P, CW], f32, tag="pt")
                for t in range(4):
                    nc.tensor.matmul(
                        pt[:, bass.ts(t, MMW)],
                        wsl(t, j),
                        rsl(t, q, t),
                        start=True,
                        stop=True,
                        tile_position=(32 * t, 0),
                    )
                # single PSUM reader: ACT copies chunk into the row image
                nc.scalar.copy(s16[:, bass.ts(q, CW)], pt[:])
            # one full-width column-accumulator update (fp16 2x mode)
            if j == 0:
                nc.vector.tensor_copy(colacc[:], s16[:])
            else:
                nc.vector.tensor_tensor(colacc[:], s16[:], colacc[:], amin)
            # row-min fold tree, one per n-block
            w0 = spool.tile([P, M // 2], f16, tag="w0")
            nc.vector.tensor_tensor(
                w0[:], s16[:, : M // 2], s16[:, M // 2 :], amin
            )
            u1 = spool.tile([P, M // 4], f16, tag="u1")
            nc.vector.tensor_tensor(
                u1[:], w0[:, : M // 4], w0[:, M // 4 :], amin
            )
            u2 = spool.tile([P, M // 8], f16, tag="u2")
            nc.vector.tensor_tensor(
                u2[:], u1[:, : M // 8], u1[:, M // 8 :], amin
            )
            u3 = spool.tile([P, M // 16], f16, tag="u3")
            nc.vector.tensor_tensor(
                u3[:], u2[:, : M // 16], u2[:, M // 16 :], amin
            )
            nc.vector.tensor_reduce(
                rmins[:, bass.ds(j, 1)],
                u3[:],
                axis=mybir.AxisListType.X,
                op=amin,
            )

        # tail: column accumulator out on the 8 SWDGE queues in parallel
        for e in range(8):
            dse = bass.ds(e * (M // 8), M // 8)
            nc.gpsimd.dma_start(col_d[:, dse], colacc[:, dse])
        nc.sync.dma_start(row_d[:], rmins[:])

    _split_multi_waits(nc)
    return nc


def _split_multi_waits(nc):
    """This toolchain's walrus encodes at most one sync wait per TPB
    instruction; hoist all but the last wait onto single-wait NOPs
    inserted just before the offending instruction (same engine queue,
    so wait ordering semantics are preserved)."""
    import copy

    from concourse import mybir

    for fn in nc.m.functions:
        for blk in fn.blocks:
            il = blk.instructions
            pos = 0
            while pos < len(il):
                inst = il[pos]
                si = inst.sync_info
                if si is not None and len(si.on_wait) > 1:
                    waits = list(si.on_wait)
                    nops = []
                    for k, w in enumerate(waits[:-1]):
                        si_n = copy.deepcopy(si)
                        si_n.on_wait = [w]
                        si_n.on_update = []
                        nop = mybir.InstNoOp(
                            name=f"{inst.name}-w{k}", engine=inst.engine
                        )
                        nop.sync_info = si_n
                        nops.append(nop)
                    si2 = copy.deepcopy(si)
                    si2.on_wait = [waits[-1]]
                    inst.sync_info = si2
                    il[pos:pos] = nops
                    pos += len(nops)
                pos += 1


def _prep_core_inputs(input1, input2):
    """Host-side augmentation; returns in_maps for the 8 cores."""
    g = np.float32(np.sqrt(SCALE))
    in_maps = []
    for c in range(NCORES):
        b, h = divmod(c, 2)
        a = np.asarray(input1[b, h * NHALF : (h + 1) * NHALF], dtype=np.float32)
        bb = np.asarray(input2[b], dtype=np.float32)
        s1 = (a * a).sum(axis=1)
        s2 = (bb * bb).sum(axis=1)
        wr = np.empty((5, NHALF + M), dtype=np.float32)
        wr[0:3, :NHALF] = -2.0 * g * a.T
        wr[3, :NHALF] = g * s1
        wr[4, :NHALF] = g
        wr[0:3, NHALF:] = g * bb.T
        wr[3, NHALF:] = g
        wr[4, NHALF:] = g * s2
        in_maps.append({"wr": wr})
    return in_maps


def _run(inputs, trace=False, tmpdir=None):
    from concourse.bass_utils import run_bass_kernel_spmd

    if "nc" not in _cache:
        _cache["nc"] = _build()
    nc = _cache["nc"]

    in_maps = _prep_core_inputs(inputs["input1"], inputs["input2"])
    res = run_bass_kernel_spmd(
        nc, in_maps, list(range(NCORES)), trace=trace, tmpdir=tmpdir
    )

    # Host-side unshard: combine per-core partial mins.
    loss = 0.0
    for b in range(B):
        rows = []
        colparts = []
        for h in range(2):
            out = res.results[2 * b + h]
            # row_out[p, j] = SCALE * min_m D^2 for n = h*NHALF + j*128 + p
            rmin = np.asarray(out["row_out"], dtype=np.float64)  # [128, 32]
            rows.append(rmin.T.reshape(-1))  # n-major: j*128 + p
            # col_out[p, m] = SCALE * min over n (h-half, n%128==p) of D^2
            cpart = np.asarray(out["col_out"], dtype=np.float64)  # [128, M]
            colparts.append(cpart.min(axis=0))
        rowmin_sq = np.concatenate(rows) / SCALE                  # [N]
        colmin_sq = np.minimum(colparts[0], colparts[1]) / SCALE  # [M]
        dist1 = np.sqrt(np.maximum(rowmin_sq, 0.0))
        dist0 = np.sqrt(np.maximum(colmin_sq, 0.0))
        loss += dist0.mean() + dist1.mean()
    loss /= B
    return np.array(loss, dtype=np.float32), res


def kernel(**inputs):
    out, _ = _run(inputs, trace=False)
    return out


# revision 24
# speedup vs baseline: 1.0231x; 1.0231x over previous
"""Chamfer distance kernel for Trainium2 (8 NeuronCores, SPMD).

Problem: input1 [B=4, N=8192, K=3], input2 [B=4, M=8192, K=3] (fp32).
  D[b,n,m] = ||input1[b,n] - input2[b,m]||
  out = mean_b( mean_m min_n D + mean_n min_m D )   (scalar fp32)

Strategy:
  - min(sqrt(x)) = sqrt(min(x)): mins on squared distances; sqrt at the end
    (host, 16k values per batch).
  - D^2 from one matmul via augmented coordinates (host-side prep),
    pre-scaled by sqrt(SCALE) so psum = SCALE * D^2 (keeps fp16 col
    accumulation clear of subnormals):
      W = g*[-2*a_x; -2*a_y; -2*a_z; ||a||^2; 1]   [5, n_half]  (stationary)
      R = g*[ b_x;    b_y;    b_z;   1; ||b||^2]   [5, M]       (moving)
      psum = W.T @ R = SCALE * D^2
  - K=5 contraction wastes 123/128 PE rows -> row-tile 4 concurrent
    matmuls via tile_position=(32g, 0); W/R data replicated on 4
    partition strips (0/32/64/96), each strip computing a different
    512-wide m-slice of a [128, 2048] psum chunk.
  - Sharding: 8 cores = 4 batches x 2 halves of N. Per core (4096 n's x
    all 8192 m's), per [128, 2048] psum chunk:
      DVE  tensor_reduce(min)  -> row-min entry       (PSUM read 1)
      ACT  copy psum -> SBUF fp16                     (PSUM read 2)
      GPS  tensor_tensor(min) fp16 -> col accumulator (SBUF)
    Host combines: partition-min + core-min + unscale + sqrt + means.
  - This walrus encodes at most ONE sync wait per TPB instruction;
    _split_multi_waits() hoists extra Tile-emitted waits onto NOPs.
"""

import numpy as np
from contextlib import ExitStack

B, N, M, K = 4, 8192, 8192, 3
NCORES = 8
NHALF = N // 2          # 4096 n's per core
P = 128                 # partitions
NB = NHALF // P         # 32 n-blocks per core
CW = 2048               # psum chunk width (4 banks, 4 row-tiled matmuls)
MC = M // CW            # 4 m-chunks
MMW = 512               # per-matmul moving width (1 bank)
SCALE = 4096.0          # psum carries SCALE * D^2

_cache = {}


def _build():
    import concourse.bass as bass
    import concourse.tile as tile
    from concourse import mybir

    f32 = mybir.dt.float32
    f16 = mybir.dt.float16
    amin = mybir.AluOpType.min
    W5 = NHALF + M  # columns of the wr operand plane

    nc = bass.Bass()
    wr_d = nc.declare_dram_parameter("wr", [5, W5], f32, isOutput=False)
    row_d = nc.declare_dram_parameter("row_out", [P, NB], f32, isOutput=True)
    col_d = nc.declare_dram_parameter("col_out", [P, M], f16, isOutput=True)

    with tile.TileContext(nc) as tc, ExitStack() as ctx:
        const = ctx.enter_context(tc.tile_pool(name="const", bufs=1))
        spool = ctx.enter_context(tc.tile_pool(name="spool", bufs=3))
        psum = ctx.enter_context(
            tc.tile_pool(name="psum", bufs=2, space="PSUM")
        )

        wr_s = const.tile([101, W5], f32)  # 4 replicas at strips 0/32/64/96
        colacc = const.tile([P, M], f16)
        rmins = const.tile([P, NB], f32)

        # load the operand plane onto all 4 partition strips straight from
        # DRAM, finely chunked so the first matmuls can start early: per
        # strip, W (cols 0:NHALF) then the 4 R chunks, spread over queues
        for g in range(4):
            nc.sync.dma_start(
                wr_s[32 * g : 32 * g + 5, :NHALF], wr_d[:, :NHALF]
            )
        for q in range(MC):
            for g in range(4):
                sl = bass.ds(NHALF + q * CW, CW)
                nc.sync.dma_start(wr_s[32 * g : 32 * g + 5, sl], wr_d[:, sl])

        def wsl(g, j):  # strip-g weights for n-block j
            return wr_s[32 * g : 32 * g + 5, bass.ts(j, P)]

        def rsl(g, q, t):  # strip-g moving operand, m-slice (q, t)
            return wr_s[32 * g : 32 * g + 5, bass.ds(NHALF + q * CW + t * MMW, MMW)]

        for j in range(NB):
            # full-width fp16 image of row-block j (filled by 4 ACT copies)
            s16 = spool.tile([P, M], f16, tag="s16")
            for q in range(MC):
                pt = psum.tile([P, CW], f32, tag="pt")
                for t in range(4):
                    nc.tensor.matmul(
                        pt[:, bass.ts(t, MMW)],
                        wsl(t, j),
                        rsl(t, q, t),
                        start=True,
                        stop=True,
                        tile_position=(32 * t, 0),
                    )
                # single PSUM reader: ACT copies chunk into the row image
                nc.scalar.copy(s16[:, bass.ts(q, CW)], pt[:])
            # one full-width column-accumulator update (fp16 2x mode)
            if j == 0:
                nc.vector.tensor_copy(colacc[:], s16[:])
            else:
                nc.vector.tensor_tensor(colacc[:], s16[:], colacc[:], amin)
            # row-min fold tree, one per n-block
            w0 = spool.tile([P, M // 2], f16, tag="w0")
            nc.vector.tensor_tensor(
                w0[:], s16[:, : M // 2], s16[:, M // 2 :], amin
            )
            u1 = spool.tile([P, M // 4], f16, tag="u1")
            nc.vector.tensor_tensor(
                u1[:], w0[:, : M // 4], w0[:, M // 4 :], amin
            )
            u2 = spool.tile([P, M // 8], f16, tag="u2")
            nc.vector.tensor_tensor(
                u2[:], u1[:, : M // 8], u1[:, M // 8 :], amin
            )
            u3 = spool.tile([P, M // 16], f16, tag="u3")
            nc.vector.tensor_tensor(
                u3[:], u2[:, : M // 16], u2[:, M // 16 :], amin
            )
            nc.vector.tensor_reduce(
                rmins[:, bass.ds(j, 1)],
                u3[:],
                axis=mybir.AxisListType.X,
                op=amin,
            )

        # tail: column accumulator out on the 8 SWDGE queues in parallel
        for e in range(8):
            dse = bass.ds(e * (M // 8), M // 8)
            nc.gpsimd.dma_start(col_d[:, dse], colacc[:, dse])
        nc.sync.dma_start(row_d[:], rmins[:])

    _split_multi_waits(nc)
    return nc


def _split_multi_waits(nc):
    """This toolchain's walrus encodes at most one sync wait per TPB
    instruction; hoist all but the last wait onto single-wait NOPs
    inserted just before the offending instruction (same engine queue,
    so wait ordering semantics are preserved)."""
    import copy

    from concourse import mybir

    for fn in nc.m.functions:
        for blk in fn.blocks:
            il = blk.instructions
            pos = 0
            while pos < len(il):
                inst = il[pos]
                si = inst.sync_info
                if si is not None and len(si.on_wait) > 1:
                    waits = list(si.on_wait)
                    nops = []
                    for k, w in enumerate(waits[:-1]):
                        si_n = copy.deepcopy(si)
                        si_n.on_wait = [w]
                        si_n.on_update = []
                        nop = mybir.InstNoOp(
                            name=f"{inst.name}-w{k}", engine=inst.engine
                        )
                        nop.sync_info = si_n
                        nops.append(nop)
                    si2 = copy.deepcopy(si)
                    si2.on_wait = [waits[-1]]
                    inst.sync_info = si2
                    il[pos:pos] = nops
                    pos += len(nops)
                pos += 1


def _prep_core_inputs(input1, input2):
    """Host-side augmentation; returns in_maps for the 8 cores."""
    g = np.float32(np.sqrt(SCALE))
    in_maps = []
    for c in range(NCORES):
        b, h = divmod(c, 2)
        a = np.asarray(input1[b, h * NHALF : (h + 1) * NHALF], dtype=np.float32)
        bb = np.asarray(input2[b], dtype=np.float32)
        s1 = (a * a).sum(axis=1)
        s2 = (bb * bb).sum(axis=1)
        wr = np.empty((5, NHALF + M), dtype=np.float32)
        wr[0:3, :NHALF] = -2.0 * g * a.T
        wr[3, :NHALF] = g * s1
        wr[4, :NHALF] = g
        wr[0:3, NHALF:] = g * bb.T
        wr[3, NHALF:] = g
        wr[4, NHALF:] = g * s2
        in_maps.append({"wr": wr})
    return in_maps


def _run(inputs, trace=False, tmpdir=None):
    from concourse.bass_utils import run_bass_kernel_spmd

    if "nc" not in _cache:
        _cache["nc"] = _build()
    nc = _cache["nc"]

    in_maps = _prep_core_inputs(inputs["input1"], inputs["input2"])
    res = run_bass_kernel_spmd(
        nc, in_maps, list(range(NCORES)), trace=trace, tmpdir=tmpdir
    )

    # Host-side unshard: combine per-core partial mins.
    loss = 0.0
    for b in range(B):
        rows = []
        colparts = []
        for h in range(2):
            out = res.results[2 * b + h]
            # row_out[p, j] = SCALE * min_m D^2 for n = h*NHALF + j*128 + p
            rmin = np.asarray(out["row_out"], dtype=np.float64)  # [128, 32]
            rows.append(rmin.T.reshape(-1))  # n-major: j*128 + p
            # col_out[p, m] = SCALE * min over n (h-half, n%128==p) of D^2
            cpart = np.asarray(out["col_out"], dtype=np.float64)  # [128, M]
            colparts.append(cpart.min(axis=0))
        rowmin_sq = np.concatenate(rows) / SCALE                  # [N]
        colmin_sq = np.minimum(colparts[0], colparts[1]) / SCALE  # [M]
        dist1 = np.sqrt(np.maximum(rowmin_sq, 0.0))
        dist0 = np.sqrt(np.maximum(colmin_sq, 0.0))
        loss += dist0.mean() + dist1.mean()
    loss /= B
    return np.array(loss, dtype=np.float32), res


def kernel(**inputs):
    out, _ = _run(inputs, trace=False)
    return out


# revision 27
# speedup vs baseline: 1.0291x; 1.0059x over previous
"""Chamfer distance kernel for Trainium2 (8 NeuronCores, SPMD).

Problem: input1 [B=4, N=8192, K=3], input2 [B=4, M=8192, K=3] (fp32).
  D[b,n,m] = ||input1[b,n] - input2[b,m]||
  out = mean_b( mean_m min_n D + mean_n min_m D )   (scalar fp32)

Strategy:
  - min(sqrt(x)) = sqrt(min(x)): mins on squared distances; sqrt at the end
    (host, 16k values per batch).
  - D^2 from one matmul via augmented coordinates (host-side prep),
    pre-scaled by sqrt(SCALE) so psum = SCALE * D^2 (keeps fp16 col
    accumulation clear of subnormals):
      W = g*[-2*a_x; -2*a_y; -2*a_z; ||a||^2; 1]   [5, n_half]  (stationary)
      R = g*[ b_x;    b_y;    b_z;   1; ||b||^2]   [5, M]       (moving)
      psum = W.T @ R = SCALE * D^2
  - K=5 contraction wastes 123/128 PE rows -> row-tile 4 concurrent
    matmuls via tile_position=(32g, 0); W/R data replicated on 4
    partition strips (0/32/64/96), each strip computing a different
    512-wide m-slice of a [128, 2048] psum chunk.
  - Sharding: 8 cores = 4 batches x 2 halves of N. Per core (4096 n's x
    all 8192 m's), per [128, 2048] psum chunk:
      DVE  tensor_reduce(min)  -> row-min entry       (PSUM read 1)
      ACT  copy psum -> SBUF fp16                     (PSUM read 2)
      GPS  tensor_tensor(min) fp16 -> col accumulator (SBUF)
    Host combines: partition-min + core-min + unscale + sqrt + means.
  - This walrus encodes at most ONE sync wait per TPB instruction;
    _split_multi_waits() hoists extra Tile-emitted waits onto NOPs.
"""

import numpy as np
from contextlib import ExitStack

B, N, M, K = 4, 8192, 8192, 3
NCORES = 8
NHALF = N // 2          # 4096 n's per core
P = 128                 # partitions
NB = NHALF // P         # 32 n-blocks per core
CW = 2048               # psum chunk width (4 banks, 4 row-tiled matmuls)
MC = M // CW            # 4 m-chunks
MMW = 512               # per-matmul moving width (1 bank)
SCALE = 4096.0          # psum carries SCALE * D^2

_cache = {}


def _build():
    import concourse.bass as bass
    import concourse.tile as tile
    from concourse import mybir

    f32 = mybir.dt.float32
    f16 = mybir.dt.float16
    amin = mybir.AluOpType.min
    W5 = NHALF + M  # columns of the wr operand plane

    nc = bass.Bass()
    wr_d = nc.declare_dram_parameter("wr", [5, W5], f32, isOutput=False)
    row_d = nc.declare_dram_parameter("row_out", [P, NB], f32, isOutput=True)
    col_d = nc.declare_dram_parameter("col_out", [P, M], f16, isOutput=True)

    with tile.TileContext(nc) as tc, ExitStack() as ctx:
        const = ctx.enter_context(tc.tile_pool(name="const", bufs=1))
        spool = ctx.enter_context(tc.tile_pool(name="spool", bufs=3))
        psum = ctx.enter_context(
            tc.tile_pool(name="psum", bufs=2, space="PSUM")
        )

        wr_s = const.tile([101, W5], f32)  # 4 replicas at strips 0/32/64/96
        colacc = const.tile([P, M], f16)
        rmins = const.tile([P, NB], f32)

        # Load the operand plane straight from DRAM. Strip g only streams
        # R columns for its own m-slices (t == g), so each strip gets W
        # (quartered by j-columns, so early matmuls start fast) plus 1/4
        # of R (strided by chunk).
        for wq in range(4):
            sl = bass.ts(wq, NHALF // 4)
            for g in range(4):
                nc.sync.dma_start(wr_s[32 * g : 32 * g + 5, sl], wr_d[:, sl])
        for g in range(4):
            for q in range(MC):
                sl = bass.ds(NHALF + q * CW + g * MMW, MMW)
                nc.sync.dma_start(wr_s[32 * g : 32 * g + 5, sl], wr_d[:, sl])

        def wsl(g, j):  # strip-g weights for n-block j
            return wr_s[32 * g : 32 * g + 5, bass.ts(j, P)]

        def rsl(g, q, t):  # strip-g moving operand, m-slice (q, t)
            return wr_s[32 * g : 32 * g + 5, bass.ds(NHALF + q * CW + t * MMW, MMW)]

        for j in range(NB):
            # full-width fp16 image of row-block j (filled by 4 ACT copies)
            s16 = spool.tile([P, M], f16, tag="s16")
            for q in range(MC):
                pt = psum.tile([P, CW], f32, tag="pt")
                for t in range(4):
                    nc.tensor.matmul(
                        pt[:, bass.ts(t, MMW)],
                        wsl(t, j),
                        rsl(t, q, t),
                        start=True,
                        stop=True,
                        tile_position=(32 * t, 0),
                    )
                # single PSUM reader: ACT copies chunk into the row image
                nc.scalar.copy(s16[:, bass.ts(q, CW)], pt[:])
            # column-accumulator update (fp16 2x mode); the last j goes
            # chunk-wise so each output DMA starts as soon as possible
            if j == 0:
                nc.vector.tensor_copy(colacc[:], s16[:])
            elif j == NB - 1:
                for q in range(MC):
                    cs = bass.ts(q, CW)
                    nc.vector.tensor_tensor(
                        colacc[:, cs], s16[:, cs], colacc[:, cs], amin
                    )
                    for half in range(2):
                        dsh = bass.ds(q * CW + half * (CW // 2), CW // 2)
                        nc.gpsimd.dma_start(col_d[:, dsh], colacc[:, dsh])
            else:
                nc.vector.tensor_tensor(colacc[:], s16[:], colacc[:], amin)
            # row-min fold tree, one per n-block
            w0 = spool.tile([P, M // 2], f16, tag="w0")
            nc.vector.tensor_tensor(
                w0[:], s16[:, : M // 2], s16[:, M // 2 :], amin
            )
            u1 = spool.tile([P, M // 4], f16, tag="u1")
            nc.vector.tensor_tensor(
                u1[:], w0[:, : M // 4], w0[:, M // 4 :], amin
            )
            u2 = spool.tile([P, M // 8], f16, tag="u2")
            nc.vector.tensor_tensor(
                u2[:], u1[:, : M // 8], u1[:, M // 8 :], amin
            )
            u3 = spool.tile([P, M // 16], f16, tag="u3")
            nc.vector.tensor_tensor(
                u3[:], u2[:, : M // 16], u2[:, M // 16 :], amin
            )
            nc.vector.tensor_reduce(
                rmins[:, bass.ds(j, 1)],
                u3[:],
                axis=mybir.AxisListType.X,
                op=amin,
            )

        nc.sync.dma_start(row_d[:], rmins[:])

    _split_multi_waits(nc)
    return nc


def _split_multi_waits(nc):
    """This toolchain's walrus encodes at most one sync wait per TPB
    instruction; hoist all but the last wait onto single-wait NOPs
    inserted just before the offending instruction (same engine queue,
    so wait ordering semantics are preserved)."""
    import copy

    from concourse import mybir

    for fn in nc.m.functions:
        for blk in fn.blocks:
            il = blk.instructions
            pos = 0
            while pos < len(il):
                inst = il[pos]
                si = inst.sync_info
                if si is not None and len(si.on_wait) > 1:
                    waits = list(si.on_wait)
                    nops = []
                    for k, w in enumerate(waits[:-1]):
                        si_n = copy.deepcopy(si)
                        si_n.on_wait = [w]
                        si_n.on_update = []
                        nop = mybir.InstNoOp(
                            name=f"{inst.name}-w{k}", engine=inst.engine
                        )
                        nop.sync_info = si_n
                        nops.append(nop)
                    si2 = copy.deepcopy(si)
                    si2.on_wait = [waits[-1]]
                    inst.sync_info = si2
                    il[pos:pos] = nops
                    pos += len(nops)
                pos += 1


def _prep_core_inputs(input1, input2):
    """Host-side augmentation; returns in_maps for the 8 cores."""
    g = np.float32(np.sqrt(SCALE))
    in_maps = []
    for c in range(NCORES):
        b, h = divmod(c, 2)
        a = np.asarray(input1[b, h * NHALF : (h + 1) * NHALF], dtype=np.float32)
        bb = np.asarray(input2[b], dtype=np.float32)
        s1 = (a * a).sum(axis=1)
        s2 = (bb * bb).sum(axis=1)
        wr = np.empty((5, NHALF + M), dtype=np.float32)
        wr[0:3, :NHALF] = -2.0 * g * a.T
        wr[3, :NHALF] = g * s1
        wr[4, :NHALF] = g
        wr[0:3, NHALF:] = g * bb.T
        wr[3, NHALF:] = g
        wr[4, NHALF:] = g * s2
        in_maps.append({"wr": wr})
    return in_maps


def _run(inputs, trace=False, tmpdir=None):
    from concourse.bass_utils import run_bass_kernel_spmd

    if "nc" not in _cache:
        _cache["nc"] = _build()
    nc = _cache["nc"]

    in_maps = _prep_core_inputs(inputs["input1"], inputs["input2"])
    res = run_bass_kernel_spmd(
        nc, in_maps, list(range(NCORES)), trace=trace, tmpdir=tmpdir
    )

    # Host-side unshard: combine per-core partial mins.
    loss = 0.0
    for b in range(B):
        rows = []
        colparts = []
        for h in range(2):
            out = res.results[2 * b + h]
            # row_out[p, j] = SCALE * min_m D^2 for n = h*NHALF + j*128 + p
            rmin = np.asarray(out["row_out"], dtype=np.float64)  # [128, 32]
            rows.append(rmin.T.reshape(-1))  # n-major: j*128 + p
            # col_out[p, m] = SCALE * min over n (h-half, n%128==p) of D^2
            cpart = np.asarray(out["col_out"], dtype=np.float64)  # [128, M]
            colparts.append(cpart.min(axis=0))
        rowmin_sq = np.concatenate(rows) / SCALE                  # [N]
        colmin_sq = np.minimum(colparts[0], colparts[1]) / SCALE  # [M]
        dist1 = np.sqrt(np.maximum(rowmin_sq, 0.0))
        dist0 = np.sqrt(np.maximum(colmin_sq, 0.0))
        loss += dist0.mean() + dist1.mean()
    loss /= B
    return np.array(loss, dtype=np.float32), res


def kernel(**inputs):
    out, _ = _run(inputs, trace=False)
    return out


# revision 29
# speedup vs baseline: 1.0511x; 1.0214x over previous
"""Chamfer distance kernel for Trainium2 (8 NeuronCores, SPMD).

Problem: input1 [B=4, N=8192, K=3], input2 [B=4, M=8192, K=3] (fp32).
  D[b,n,m] = ||input1[b,n] - input2[b,m]||
  out = mean_b( mean_m min_n D + mean_n min_m D )   (scalar fp32)

Strategy:
  - min(sqrt(x)) = sqrt(min(x)): mins on squared distances; sqrt at the end
    (host, 16k values per batch).
  - D^2 from one matmul via augmented coordinates (host-side prep),
    pre-scaled by sqrt(SCALE) so psum = SCALE * D^2 (keeps fp16 col
    accumulation clear of subnormals):
      W = g*[-2*a_x; -2*a_y; -2*a_z; ||a||^2; 1]   [5, n_half]  (stationary)
      R = g*[ b_x;    b_y;    b_z;   1; ||b||^2]   [5, M]       (moving)
      psum = W.T @ R = SCALE * D^2
  - K=5 contraction wastes 123/128 PE rows -> row-tile 4 concurrent
    matmuls via tile_position=(32g, 0); W/R data replicated on 4
    partition strips (0/32/64/96), each strip computing a different
    512-wide m-slice of a [128, 2048] psum chunk.
  - Sharding: 8 cores = 4 batches x 2 halves of N. Per core (4096 n's x
    all 8192 m's), per [128, 2048] psum chunk:
      DVE  tensor_reduce(min)  -> row-min entry       (PSUM read 1)
      ACT  copy psum -> SBUF fp16                     (PSUM read 2)
      GPS  tensor_tensor(min) fp16 -> col accumulator (SBUF)
    Host combines: partition-min + core-min + unscale + sqrt + means.
  - This walrus encodes at most ONE sync wait per TPB instruction;
    _split_multi_waits() hoists extra Tile-emitted waits onto NOPs.
"""

import numpy as np
from contextlib import ExitStack

B, N, M, K = 4, 8192, 8192, 3
NCORES = 8
NHALF = N // 2          # 4096 n's per core
P = 128                 # partitions
NB = NHALF // P         # 32 n-blocks per core
CW = 2048               # psum chunk width (4 banks, 4 row-tiled matmuls)
MC = M // CW            # 4 m-chunks
MMW = 512               # per-matmul moving width (1 bank)
SCALE = 4096.0          # psum carries SCALE * D^2

_cache = {}


def _build():
    import concourse.bass as bass
    import concourse.tile as tile
    from concourse import mybir

    f32 = mybir.dt.float32
    f16 = mybir.dt.float16
    amin = mybir.AluOpType.min
    W5 = NHALF + M  # columns of the wr operand plane

    nc = bass.Bass()
    wr_d = nc.declare_dram_parameter("wr", [5, W5], f32, isOutput=False)
    row_d = nc.declare_dram_parameter("row_out", [P, NB], f32, isOutput=True)
    col_d = nc.declare_dram_parameter("col_out", [P, M], f16, isOutput=True)

    with tile.TileContext(nc) as tc, ExitStack() as ctx:
        const = ctx.enter_context(tc.tile_pool(name="const", bufs=1))
        spool = ctx.enter_context(tc.tile_pool(name="spool", bufs=3))
        psum = ctx.enter_context(
            tc.tile_pool(name="psum", bufs=2, space="PSUM")
        )

        wr_s = const.tile([101, W5], f32)  # 4 replicas at strips 0/32/64/96
        colacc = const.tile([P, M], f16)
        rmins = const.tile([P, NB], f32)

        # Load the operand plane straight from DRAM: 8 chunky DMAs (one
        # per HWDGE queue). Strip g needs W (all j's) but only its own
        # R m-slices (t == g) -> one strided DMA covers R/4 per strip.
        for g in range(4):
            nc.sync.dma_start(
                wr_s[32 * g : 32 * g + 5, :NHALF], wr_d[:, :NHALF]
            )
        for g in range(4):
            gsl = bass.ts(g, MMW)
            rv_out = wr_s[32 * g : 32 * g + 5, NHALF:].rearrange(
                "p (q c) -> p q c", q=MC
            )[:, :, gsl]
            rv_in = wr_d[:, NHALF:].rearrange("p (q c) -> p q c", q=MC)[
                :, :, gsl
            ]
            nc.sync.dma_start(rv_out, rv_in)

        def wsl(g, j):  # strip-g weights for n-block j
            return wr_s[32 * g : 32 * g + 5, bass.ts(j, P)]

        def rsl(g, q, t):  # strip-g moving operand, m-slice (q, t)
            return wr_s[32 * g : 32 * g + 5, bass.ds(NHALF + q * CW + t * MMW, MMW)]

        for j in range(NB):
            # full-width fp16 image of row-block j (filled by 4 ACT copies)
            s16 = spool.tile([P, M], f16, tag="s16")
            for q in range(MC):
                pt = psum.tile([P, CW], f32, tag="pt")
                for t in range(4):
                    nc.tensor.matmul(
                        pt[:, bass.ts(t, MMW)],
                        wsl(t, j),
                        rsl(t, q, t),
                        start=True,
                        stop=True,
                        tile_position=(32 * t, 0),
                    )
                # single PSUM reader: ACT copies chunk into the row image
                nc.scalar.copy(s16[:, bass.ts(q, CW)], pt[:])
            # column-accumulator update (fp16 2x mode); the last j goes
            # chunk-wise so each output DMA starts as soon as possible
            if j == 0:
                nc.vector.tensor_copy(colacc[:], s16[:])
            elif j == NB - 1:
                for q in range(MC):
                    cs = bass.ts(q, CW)
                    nc.vector.tensor_tensor(
                        colacc[:, cs], s16[:, cs], colacc[:, cs], amin
                    )
                    for half in range(2):
                        dsh = bass.ds(q * CW + half * (CW // 2), CW // 2)
                        nc.gpsimd.dma_start(col_d[:, dsh], colacc[:, dsh])
            else:
                nc.vector.tensor_tensor(colacc[:], s16[:], colacc[:], amin)
            # row-min fold tree, one per n-block
            w0 = spool.tile([P, M // 2], f16, tag="w0")
            nc.vector.tensor_tensor(
                w0[:], s16[:, : M // 2], s16[:, M // 2 :], amin
            )
            u1 = spool.tile([P, M // 4], f16, tag="u1")
            nc.vector.tensor_tensor(
                u1[:], w0[:, : M // 4], w0[:, M // 4 :], amin
            )
            u2 = spool.tile([P, M // 8], f16, tag="u2")
            nc.vector.tensor_tensor(
                u2[:], u1[:, : M // 8], u1[:, M // 8 :], amin
            )
            u3 = spool.tile([P, M // 16], f16, tag="u3")
            nc.vector.tensor_tensor(
                u3[:], u2[:, : M // 16], u2[:, M // 16 :], amin
            )
            nc.vector.tensor_reduce(
                rmins[:, bass.ds(j, 1)],
                u3[:],
                axis=mybir.AxisListType.X,
                op=amin,
            )

        nc.sync.dma_start(row_d[:], rmins[:])

    _split_multi_waits(nc)
    return nc


def _split_multi_waits(nc):
    """This toolchain's walrus encodes at most one sync wait per TPB
    instruction; hoist all but the last wait onto single-wait NOPs
    inserted just before the offending instruction (same engine queue,
    so wait ordering semantics are preserved)."""
    import copy

    from concourse import mybir

    for fn in nc.m.functions:
        for blk in fn.blocks:
            il = blk.instructions
            pos = 0
            while pos < len(il):
                inst = il[pos]
                si = inst.sync_info
                if si is not None and len(si.on_wait) > 1:
                    waits = list(si.on_wait)
                    nops = []
                    for k, w in enumerate(waits[:-1]):
                        si_n = copy.deepcopy(si)
                        si_n.on_wait = [w]
                        si_n.on_update = []
                        nop = mybir.InstNoOp(
                            name=f"{inst.name}-w{k}", engine=inst.engine
                        )
                        nop.sync_info = si_n
                        nops.append(nop)
                    si2 = copy.deepcopy(si)
                    si2.on_wait = [waits[-1]]
                    inst.sync_info = si2
                    il[pos:pos] = nops
                    pos += len(nops)
                pos += 1


def _prep_core_inputs(input1, input2):
    """Host-side augmentation; returns in_maps for the 8 cores."""
    g = np.float32(np.sqrt(SCALE))
    in_maps = []
    for c in range(NCORES):
        b, h = divmod(c, 2)
        a = np.asarray(input1[b, h * NHALF : (h + 1) * NHALF], dtype=np.float32)
        bb = np.asarray(input2[b], dtype=np.float32)
        s1 = (a * a).sum(axis=1)
        s2 = (bb * bb).sum(axis=1)
        wr = np.empty((5, NHALF + M), dtype=np.float32)
        wr[0:3, :NHALF] = -2.0 * g * a.T
        wr[3, :NHALF] = g * s1
        wr[4, :NHALF] = g
        wr[0:3, NHALF:] = g * bb.T
        wr[3, NHALF:] = g
        wr[4, NHALF:] = g * s2
        in_maps.append({"wr": wr})
    return in_maps


def _run(inputs, trace=False, tmpdir=None):
    from concourse.bass_utils import run_bass_kernel_spmd

    if "nc" not in _cache:
        _cache["nc"] = _build()
    nc = _cache["nc"]

    in_maps = _prep_core_inputs(inputs["input1"], inputs["input2"])
    res = run_bass_kernel_spmd(
        nc, in_maps, list(range(NCORES)), trace=trace, tmpdir=tmpdir
    )

    # Host-side unshard: combine per-core partial mins.
    loss = 0.0
    for b in range(B):
        rows = []
        colparts = []
        for h in range(2):
            out = res.results[2 * b + h]
            # row_out[p, j] = SCALE * min_m D^2 for n = h*NHALF + j*128 + p
            rmin = np.asarray(out["row_out"], dtype=np.float64)  # [128, 32]
            rows.append(rmin.T.reshape(-1))  # n-major: j*128 + p
            # col_out[p, m] = SCALE * min over n (h-half, n%128==p) of D^2
            cpart = np.asarray(out["col_out"], dtype=np.float64)  # [128, M]
            colparts.append(cpart.min(axis=0))
        rowmin_sq = np.concatenate(rows) / SCALE                  # [N]
        colmin_sq = np.minimum(colparts[0], colparts[1]) / SCALE  # [M]
        dist1 = np.sqrt(np.maximum(rowmin_sq, 0.0))
        dist0 = np.sqrt(np.maximum(colmin_sq, 0.0))
        loss += dist0.mean() + dist1.mean()
    loss /= B
    return np.array(loss, dtype=np.float32), res


def kernel(**inputs):
    out, _ = _run(inputs, trace=False)
    return out


# revision 30
# speedup vs baseline: 1.0641x; 1.0124x over previous
"""Chamfer distance kernel for Trainium2 (8 NeuronCores, SPMD).

Problem: input1 [B=4, N=8192, K=3], input2 [B=4, M=8192, K=3] (fp32).
  D[b,n,m] = ||input1[b,n] - input2[b,m]||
  out = mean_b( mean_m min_n D + mean_n min_m D )   (scalar fp32)

Strategy:
  - min(sqrt(x)) = sqrt(min(x)): mins on squared distances; sqrt at the end
    (host, 16k values per batch).
  - D^2 from one matmul via augmented coordinates (host-side prep),
    pre-scaled by sqrt(SCALE) so psum = SCALE * D^2 (keeps fp16 col
    accumulation clear of subnormals):
      W = g*[-2*a_x; -2*a_y; -2*a_z; ||a||^2; 1]   [5, n_half]  (stationary)
      R = g*[ b_x;    b_y;    b_z;   1; ||b||^2]   [5, M]       (moving)
      psum = W.T @ R = SCALE * D^2
  - K=5 contraction wastes 123/128 PE rows -> row-tile 4 concurrent
    matmuls via tile_position=(32g, 0); W/R data replicated on 4
    partition strips (0/32/64/96), each strip computing a different
    512-wide m-slice of a [128, 2048] psum chunk.
  - Sharding: 8 cores = 4 batches x 2 halves of N. Per core (4096 n's x
    all 8192 m's), per [128, 2048] psum chunk:
      DVE  tensor_reduce(min)  -> row-min entry       (PSUM read 1)
      ACT  copy psum -> SBUF fp16                     (PSUM read 2)
      GPS  tensor_tensor(min) fp16 -> col accumulator (SBUF)
    Host combines: partition-min + core-min + unscale + sqrt + means.
  - This walrus encodes at most ONE sync wait per TPB instruction;
    _split_multi_waits() hoists extra Tile-emitted waits onto NOPs.
"""

import numpy as np
from contextlib import ExitStack

B, N, M, K = 4, 8192, 8192, 3
NCORES = 8
NHALF = N // 2          # 4096 n's per core
P = 128                 # partitions
NB = NHALF // P         # 32 n-blocks per core
CW = 2048               # psum chunk width (4 banks, 4 row-tiled matmuls)
MC = M // CW            # 4 m-chunks
MMW = 512               # per-matmul moving width (1 bank)
SCALE = 4096.0          # psum carries SCALE * D^2

_cache = {}


def _build():
    import concourse.bass as bass
    import concourse.tile as tile
    from concourse import mybir

    f32 = mybir.dt.float32
    f16 = mybir.dt.float16
    amin = mybir.AluOpType.min
    W5 = NHALF + M  # columns of the wr operand plane

    nc = bass.Bass()
    wr_d = nc.declare_dram_parameter("wr", [5, W5], f32, isOutput=False)
    row_d = nc.declare_dram_parameter("row_out", [P, NB], f32, isOutput=True)
    col_d = nc.declare_dram_parameter("col_out", [P, M], f16, isOutput=True)

    with tile.TileContext(nc) as tc, ExitStack() as ctx:
        const = ctx.enter_context(tc.tile_pool(name="const", bufs=1))
        spool = ctx.enter_context(tc.tile_pool(name="spool", bufs=3))
        psum = ctx.enter_context(
            tc.tile_pool(name="psum", bufs=2, space="PSUM")
        )

        wr_s = const.tile([101, W5], f32)  # 4 replicas at strips 0/32/64/96
        colacc = const.tile([P, M], f16)
        rmins = const.tile([P, NB], f32)

        # Load the operand plane straight from DRAM: 8 chunky DMAs (one
        # per HWDGE queue). Strip g needs W (all j's) but only its own
        # R m-slices (t == g) -> one strided DMA covers R/4 per strip.
        WQ = NHALF // 4
        for g in range(4):  # first W quarter (j=0..7) -> queues 0-3
            nc.sync.dma_start(
                wr_s[32 * g : 32 * g + 5, :WQ], wr_d[:, :WQ]
            )
        for g in range(4):  # strip-local R -> queues 4-7
            gsl = bass.ts(g, MMW)
            rv_out = wr_s[32 * g : 32 * g + 5, NHALF:].rearrange(
                "p (q c) -> p q c", q=MC
            )[:, :, gsl]
            rv_in = wr_d[:, NHALF:].rearrange("p (q c) -> p q c", q=MC)[
                :, :, gsl
            ]
            nc.sync.dma_start(rv_out, rv_in)
        for wq in range(1, 4):  # remaining W quarters stream in behind
            sl = bass.ts(wq, WQ)
            for g in range(4):
                nc.sync.dma_start(wr_s[32 * g : 32 * g + 5, sl], wr_d[:, sl])

        def wsl(g, j):  # strip-g weights for n-block j
            return wr_s[32 * g : 32 * g + 5, bass.ts(j, P)]

        def rsl(g, q, t):  # strip-g moving operand, m-slice (q, t)
            return wr_s[32 * g : 32 * g + 5, bass.ds(NHALF + q * CW + t * MMW, MMW)]

        for j in range(NB):
            # full-width fp16 image of row-block j (filled by 4 ACT copies)
            s16 = spool.tile([P, M], f16, tag="s16")
            for q in range(MC):
                pt = psum.tile([P, CW], f32, tag="pt")
                for t in range(4):
                    nc.tensor.matmul(
                        pt[:, bass.ts(t, MMW)],
                        wsl(t, j),
                        rsl(t, q, t),
                        start=True,
                        stop=True,
                        tile_position=(32 * t, 0),
                    )
                # single PSUM reader: ACT copies chunk into the row image
                nc.scalar.copy(s16[:, bass.ts(q, CW)], pt[:])
            # column-accumulator update (fp16 2x mode); the last j goes
            # chunk-wise so each output DMA starts as soon as possible
            if j == 0:
                nc.vector.tensor_copy(colacc[:], s16[:])
            elif j == NB - 1:
                for q in range(MC):
                    cs = bass.ts(q, CW)
                    nc.vector.tensor_tensor(
                        colacc[:, cs], s16[:, cs], colacc[:, cs], amin
                    )
                    for half in range(2):
                        dsh = bass.ds(q * CW + half * (CW // 2), CW // 2)
                        nc.gpsimd.dma_start(col_d[:, dsh], colacc[:, dsh])
            else:
                nc.vector.tensor_tensor(colacc[:], s16[:], colacc[:], amin)
            # row-min fold tree, one per n-block
            w0 = spool.tile([P, M // 2], f16, tag="w0")
            nc.vector.tensor_tensor(
                w0[:], s16[:, : M // 2], s16[:, M // 2 :], amin
            )
            u1 = spool.tile([P, M // 4], f16, tag="u1")
            nc.vector.tensor_tensor(
                u1[:], w0[:, : M // 4], w0[:, M // 4 :], amin
            )
            u2 = spool.tile([P, M // 8], f16, tag="u2")
            nc.vector.tensor_tensor(
                u2[:], u1[:, : M // 8], u1[:, M // 8 :], amin
            )
            u3 = spool.tile([P, M // 16], f16, tag="u3")
            nc.vector.tensor_tensor(
                u3[:], u2[:, : M // 16], u2[:, M // 16 :], amin
            )
            nc.vector.tensor_reduce(
                rmins[:, bass.ds(j, 1)],
                u3[:],
                axis=mybir.AxisListType.X,
                op=amin,
            )

        nc.sync.dma_start(row_d[:], rmins[:])

    _split_multi_waits(nc)
    return nc


def _split_multi_waits(nc):
    """This toolchain's walrus encodes at most one sync wait per TPB
    instruction; hoist all but the last wait onto single-wait NOPs
    inserted just before the offending instruction (same engine queue,
    so wait ordering semantics are preserved)."""
    import copy

    from concourse import mybir

    for fn in nc.m.functions:
        for blk in fn.blocks:
            il = blk.instructions
            pos = 0
            while pos < len(il):
                inst = il[pos]
                si = inst.sync_info
                if si is not None and len(si.on_wait) > 1:
                    waits = list(si.on_wait)
                    nops = []
                    for k, w in enumerate(waits[:-1]):
                        si_n = copy.deepcopy(si)
                        si_n.on_wait = [w]
                        si_n.on_update = []
                        nop = mybir.InstNoOp(
                            name=f"{inst.name}-w{k}", engine=inst.engine
                        )
                        nop.sync_info = si_n
                        nops.append(nop)
                    si2 = copy.deepcopy(si)
                    si2.on_wait = [waits[-1]]
                    inst.sync_info = si2
                    il[pos:pos] = nops
                    pos += len(nops)
                pos += 1


def _prep_core_inputs(input1, input2):
    """Host-side augmentation; returns in_maps for the 8 cores."""
    g = np.float32(np.sqrt(SCALE))
    in_maps = []
    for c in range(NCORES):
        b, h = divmod(c, 2)
        a = np.asarray(input1[b, h * NHALF : (h + 1) * NHALF], dtype=np.float32)
        bb = np.asarray(input2[b], dtype=np.float32)
        s1 = (a * a).sum(axis=1)
        s2 = (bb * bb).sum(axis=1)
        wr = np.empty((5, NHALF + M), dtype=np.float32)
        wr[0:3, :NHALF] = -2.0 * g * a.T
        wr[3, :NHALF] = g * s1
        wr[4, :NHALF] = g
        wr[0:3, NHALF:] = g * bb.T
        wr[3, NHALF:] = g
        wr[4, NHALF:] = g * s2
        in_maps.append({"wr": wr})
    return in_maps


def _run(inputs, trace=False, tmpdir=None):
    from concourse.bass_utils import run_bass_kernel_spmd

    if "nc" not in _cache:
        _cache["nc"] = _build()
    nc = _cache["nc"]

    in_maps = _prep_core_inputs(inputs["input1"], inputs["input2"])
    res = run_bass_kernel_spmd(
        nc, in_maps, list(range(NCORES)), trace=trace, tmpdir=tmpdir
    )

    # Host-side unshard: combine per-core partial mins.
    loss = 0.0
    for b in range(B):
        rows = []
        colparts = []
        for h in range(2):
            out = res.results[2 * b + h]
            # row_out[p, j] = SCALE * min_m D^2 for n = h*NHALF + j*128 + p
            rmin = np.asarray(out["row_out"], dtype=np.float64)  # [128, 32]
            rows.append(rmin.T.reshape(-1))  # n-major: j*128 + p
            # col_out[p, m] = SCALE * min over n (h-half, n%128==p) of D^2
            cpart = np.asarray(out["col_out"], dtype=np.float64)  # [128, M]
            colparts.append(cpart.min(axis=0))
        rowmin_sq = np.concatenate(rows) / SCALE                  # [N]
        colmin_sq = np.minimum(colparts[0], colparts[1]) / SCALE  # [M]
        dist1 = np.sqrt(np.maximum(rowmin_sq, 0.0))
        dist0 = np.sqrt(np.maximum(colmin_sq, 0.0))
        loss += dist0.mean() + dist1.mean()
    loss /= B
    return np.array(loss, dtype=np.float32), res


def kernel(**inputs):
    out, _ = _run(inputs, trace=False)
    return out
